# revision 39
# baseline (speedup 1.0000x reference)
"""AttentionFlow Trainium2 Bass kernel (v5, bf16 + fully-inline batch tails).

Math (per batch):
  d = 256; w = [w_c | w_q | w_m]
  sim[t,j] = s_c[t] + s_q[j] + sum_d C[t,d] w_m[d] Q[j,d]   (+b, which cancels)
  attn = softmax_j(sim);  AQ = attn @ Q
  beta = softmax_t(max_j sim);  AC = beta @ C
  out = concat([C, AQ, C*AQ, C*AC], axis=-1)

Sharding: data-parallel over batch B=32 across 8 NeuronCores (4 batches/core).

Design (v5):
  - bf16 internal pipeline: fp32 only in matmul PSUM accumulation, ACT
    scale/bias columns, and the HBM output (SWDGE cast-DMA bf16->fp32).
    Single-pass PE transposes, FWL weight loads, 2x DVE tensor ops,
    half SBUF traffic.
  - w_c folded into the g stationary: qwc = Q^T*wm + wc (one DVE
    tensor_scalar from the Q^T PSUM), so sim = qwc^T @ C^T + s_q and the
    separate s_c matmuls are gone.  s_q enters as the exp bias column.
  - AC matmuls read the staged C directly (no [C|1] copy); s_tot comes
    from a 1-col matmul ones^T @ n_all reduced on DVE.  The AC->beta
    broadcast acb is a PE rank-1 matmul (ones_row x ac_row), not gpsimd.
  - The whole batch tail is INLINE: AC starts per colmax chunk, the
    s_tot/acb chain overlaps the U loop, o4 + the output DMA for the
    first half fire mid-U-loop.  Output DMA of batch b now streams
    during body b (v2-v4 deferred it into body b+1, which left the DMA
    system starved for the first ~35us and draining ~15us at the end).
  - Software pipelining for PE: the NEXT batch's Q^T transposes are
    emitted before the AC group (covering the colmax-reduce latency) and
    its C^T transposes after the U loop (covering the o4/DMA drain), so
    the in-order PE queue always has ready work.
  - Permuted t-layout t = 8p + i (partition-major): per-t math is
    permutation-invariant, T-sums are order-free, and each output DMA
    gets 16KB contiguous DRAM lines per partition.
"""

import numpy as np

import concourse.bass as bass
import concourse.mybir as mybir
import concourse.tile as tile
from concourse import bacc
from concourse.bass_utils import run_bass_kernel_spmd
from concourse.masks import make_identity

F32 = mybir.dt.float32
BF16 = mybir.dt.bfloat16
AF = mybir.ActivationFunctionType
ALU = mybir.AluOpType
AX = mybir.AxisListType

B, T, J, D = 32, 1024, 128, 256
NCORES = 8
BPC = B // NCORES      # batches per core
NT = T // 128          # t-tiles per batch
ND = D // 128          # d-tiles
NCH = T // 512         # 512-wide chunks per batch
TPC = 4                # t-tiles per chunk


def _bcast_row(ap_1d, nparts):
    """DRAM AP [n] -> [nparts, n] with partition stride 0 (DMA broadcast)."""
    return bass.AP(
        tensor=ap_1d.tensor, offset=ap_1d.offset, ap=[[0, nparts]] + list(ap_1d.ap)
    )


def _tile_bcast(ap_2d, reps):
    """[128, n] AP -> [128, reps, n] with 0-stride middle dim."""
    a = list(ap_2d.ap)
    return bass.AP(
        tensor=ap_2d.tensor, offset=ap_2d.offset,
        ap=[a[0], [0, reps]] + a[1:],
    )


def build_nc():
    nc = bacc.Bacc()
    ctx_in = nc.declare_dram_parameter("context", [BPC, T, D], F32, isOutput=False)
    qry_in = nc.declare_dram_parameter("query", [BPC, J, D], F32, isOutput=False)
    w_in = nc.declare_dram_parameter("w", [3 * D], F32, isOutput=False)
    out_ext = nc.declare_dram_parameter("out", [BPC, T, 4 * D], F32, isOutput=True)

    with tile.TileContext(nc) as tc:
        _body(tc, ctx_in, qry_in, w_in, out_ext)
    nc.finalize()
    return nc


def _body(tc, ctx_in, qry_in, w_in, out_ext):
    nc = tc.nc
    from contextlib import ExitStack

    with ExitStack() as ctx:
        consts = ctx.enter_context(tc.tile_pool(name="consts", bufs=1))
        stage_p = ctx.enter_context(tc.tile_pool(name="stage", bufs=4))
        big = ctx.enter_context(tc.tile_pool(name="big", bufs=2))
        work = ctx.enter_context(tc.tile_pool(name="work", bufs=2))
        tmp = ctx.enter_context(tc.tile_pool(name="tmp", bufs=2))
        # PSUM budget (8 banks): tr 2 + g 2 + u 3 + ac 1 = 8
        ps_tr = ctx.enter_context(tc.tile_pool(name="ps_tr", bufs=2, space="PSUM"))
        ps_g = ctx.enter_context(tc.tile_pool(name="ps_g", bufs=2, space="PSUM"))
        ps_u = ctx.enter_context(tc.tile_pool(name="ps_u", bufs=3, space="PSUM"))
        ps_ac = ctx.enter_context(tc.tile_pool(name="ps_ac", bufs=1, space="PSUM"))

        # --- identity first: it gates the first Q^T the moment Q lands ---
        ident = consts.tile([128, 128], BF16)
        make_identity(nc, ident)

        def warmup(n):
            """Back-to-back junk matmuls while the PE waits on input DMAs:
            flips the HAM clock gate to K=8/8 (~3.4us of sustained activity)
            before the first real matmul.  Transposes don't count as PE
            activity for HAM, so without this the whole first batch runs at
            half clock."""
            wu_ps = ps_g.tile([128, 128], F32, tag="g")
            for _ in range(n):
                nc.tensor.matmul(wu_ps, lhsT=ident, rhs=ident)

        loads = {}

        def load_batch(bb, nsplit=1):
            st = stage_p.tile([128, NT, 4 * D], BF16, tag="stage")
            qa = work.tile([128, D + 2], BF16, tag="Qaug")
            # Q first: it gates the first PE work (Q^T transposes).
            # SWDGE cast-DMA: fp32 DRAM -> bf16 SBUF.
            nc.gpsimd.dma_start(out=qa[:, 0:D], in_=qry_in[bb])
            nc.vector.memset(qa[:, D : D + 2], 1.0)
            src = ctx_in[bb].rearrange("(p i) d -> p i d", i=NT)
            step = NT // nsplit
            for s in range(nsplit):
                nc.gpsimd.dma_start(
                    out=st[:, s * step : (s + 1) * step, 0:D],
                    in_=src[:, s * step : (s + 1) * step, :],
                )
            # For the first two batches, the C column of the output depends
            # only on this load: stream it out now (cast bf16->fp32) to fill
            # the otherwise-idle DMA window while batch 0 computes.  Later
            # batches keep full 16KB-line row DMAs (higher HBM efficiency)
            # since the DMA queue is saturated by then anyway.
            if bb < 2:
                out_r = out_ext[bb].rearrange("(p i) d -> p i d", i=NT)
                nc.gpsimd.dma_start(out=out_r[:, :, 0:D], in_=st[:, :, 0:D])
            loads[bb] = (st, qa)

        load_batch(0, nsplit=2)

        # --- remaining constants ---
        ones_col = consts.tile([128, 1], BF16)
        nc.vector.memset(ones_col, 1.0)
        ones_row = consts.tile([1, 128], BF16)
        nc.vector.memset(ones_row, 1.0)
        # w_c / w_m as per-partition fp32 columns; no cast -> HWDGE
        wc_raw = consts.tile([128, ND], F32)
        wm_cols = consts.tile([128, ND], F32)
        for k in range(ND):
            nc.sync.dma_start(
                out=wc_raw[:, k : k + 1],
                in_=w_in[k * 128 : (k + 1) * 128].rearrange("(p o) -> p o", o=1),
            )
            nc.sync.dma_start(
                out=wm_cols[:, k : k + 1],
                in_=w_in[2 * D + k * 128 : 2 * D + (k + 1) * 128].rearrange(
                    "(p o) -> p o", o=1
                ),
            )
        # w_q broadcast to all partitions, cast to bf16
        wq_b = consts.tile([128, D], BF16)
        nc.gpsimd.dma_start(out=wq_b, in_=_bcast_row(w_in[D : 2 * D], 128))

        load_batch(1)

        warmup(32)

        pre = {}

        def qt_phase(bb):
            """Q^T -> qwc (+ wc bias) and the s_q column for batch bb."""
            _, qa = loads[bb]
            qt_ps = ps_tr.tile([128, D], BF16, tag="tr")
            for k in range(ND):
                nc.tensor.transpose(
                    qt_ps[:, k * 128 : (k + 1) * 128],
                    qa[:, k * 128 : (k + 1) * 128],
                    ident,
                )
            qwc = work.tile([128, D], BF16, tag="qwc")
            for k in range(ND):
                nc.vector.tensor_scalar(
                    out=qwc[:, k * 128 : (k + 1) * 128],
                    in0=qt_ps[:, k * 128 : (k + 1) * 128],
                    scalar1=wm_cols[:, k : k + 1],
                    scalar2=wc_raw[:, k : k + 1],
                    op0=ALU.mult,
                    op1=ALU.add,
                )
            sq_scr = tmp.tile([128, D], BF16, tag="sq_scr")
            sq_col = work.tile([128, 1], F32, tag="sq_col")
            nc.vector.tensor_mul(sq_scr, qa[:, 0:D], wq_b)
            nc.vector.reduce_sum(out=sq_col, in_=sq_scr, axis=AX.X)
            pre.setdefault(bb, {})["qwc"] = qwc
            pre[bb]["sq_col"] = sq_col

        def ct_step(bb, i2):
            """One pair of C^T tile transposes + the PSUM->SBUF copy."""
            st, _ = loads[bb]
            ctT = pre[bb]["ctT"]
            ct_ps = ps_tr.tile([128, 2 * ND * 128], BF16, tag="tr")
            for u in range(2):
                i = 2 * i2 + u
                for k in range(ND):
                    nc.tensor.transpose(
                        ct_ps[:, (2 * u + k) * 128 : (2 * u + k + 1) * 128],
                        st[:, i, k * 128 : (k + 1) * 128],
                        ident,
                    )
            dst = ctT[:, :, 2 * i2 : 2 * i2 + 2, :]
            srcv = ct_ps.rearrange("p (t k x) -> p k t x", t=2, k=ND)
            nc.scalar.copy(dst, srcv)

        def ct_phase(bb):
            """C^T via PE transposes (bf16, single-pass) + PSUM->SBUF copies."""
            ctT = big.tile([128, ND, NT, 128], BF16, tag="ctT")
            pre.setdefault(bb, {})["ctT"] = ctT
            for i2 in range(NT // 2):
                ct_step(bb, i2)

        def g_phase(bb):
            """g matmuls for batch bb (w_c already folded into qwc)."""
            qwc, ctT = pre[bb]["qwc"], pre[bb]["ctT"]
            g_list = []
            for c in range(NCH):
                g_ps = ps_g.tile([128, 512], F32, tag="g")
                for k in range(ND):
                    nc.tensor.matmul(
                        g_ps,
                        lhsT=qwc[:, k * 128 : (k + 1) * 128],
                        rhs=ctT[:, k, TPC * c : TPC * (c + 1), :],
                        start=(k == 0),
                        stop=(k == ND - 1),
                    )
                g_list.append(g_ps)
            pre[bb]["g"] = g_list

        qt_phase(0)
        warmup(16)
        ct_phase(0)
        g_phase(0)

        for b in range(BPC):
            if b + 2 < BPC:
                load_batch(b + 2)
            st, Qaug = loads.pop(b)
            ph = pre.pop(b)
            sq_col, g_list = ph["sq_col"], ph["g"]
            out_r = out_ext[b].rearrange("(p i) d -> p i d", i=NT)
            nxt = b + 1 < BPC

            ET = big.tile([128, T], BF16, tag="ET")
            n_all = work.tile([128, NT], BF16, tag="n_all")

            # ---- per chunk: E = exp(g + s_q); colmax via PE transposes ----
            for c in range(NCH):
                nc.scalar.activation(
                    ET[:, c * 512 : (c + 1) * 512], g_list[c], AF.Exp, bias=sq_col
                )
                tp_ps = ps_tr.tile([128, TPC, 128], BF16, tag="tr")
                for i in range(TPC):
                    t0 = (TPC * c + i) * 128
                    nc.tensor.transpose(
                        tp_ps[:, i, :], ET[:, t0 : t0 + 128], ident
                    )
                nc.vector.reduce_max(
                    out=n_all[:, TPC * c : TPC * (c + 1)], in_=tp_ps, axis=AX.X
                )

            # ---- next batch's Q^T here: ready PE work covering the
            #      colmax-reduce -> AC dependency latency ----
            if nxt:
                qt_phase(b + 1)
                ctT_n = big.tile([128, ND, NT, 128], BF16, tag="ctT")
                pre.setdefault(b + 1, {})["ctT"] = ctT_n

            # ---- beta numerator: AC = sum_t n[t] C[t,:] ----
            ac_ps = ps_ac.tile([1, D], F32, tag="ac")
            for ii in range(NT):
                nc.tensor.matmul(
                    ac_ps,
                    lhsT=n_all[:, ii : ii + 1],
                    rhs=st[:, ii, 0:D],
                    start=(ii == 0),
                    stop=(ii == NT - 1),
                )
            # s_tot = sum_t n[t] via ones^T @ n_all, then 1/s_tot
            s_ps = ps_u.tile([1, NT], F32, tag="u")
            nc.tensor.matmul(s_ps, lhsT=ones_col, rhs=n_all)
            s_tot = work.tile([1, 1], F32, tag="s_tot")
            nc.vector.reduce_sum(out=s_tot, in_=s_ps, axis=AX.X)
            r_s = work.tile([1, 1], F32, tag="r_s")
            nc.vector.reciprocal(r_s, s_tot)

            # ---- U loop first half (tiles 0..3), chain resolves meanwhile --
            r_all = work.tile([128, NT], F32, tag="r_all")
            acb = None

            def u_tile(i):
                u_ps = ps_u.tile([128, D + 2], F32, tag="u")
                nc.tensor.matmul(
                    u_ps, lhsT=ET[:, i * 128 : (i + 1) * 128], rhs=Qaug
                )
                nc.vector.reciprocal(r_all[:, i : i + 1], u_ps[:, D : D + 1])
                # aq = U/r, alternating ACT/DVE to balance the engines
                if i % 2 == 0:
                    nc.scalar.activation(
                        st[:, i, D : 2 * D], u_ps[:, 0:D], AF.Copy,
                        scale=r_all[:, i : i + 1],
                    )
                else:
                    nc.vector.tensor_scalar_mul(
                        st[:, i, D : 2 * D], u_ps[:, 0:D], r_all[:, i : i + 1]
                    )
                if i % TPC == TPC - 1:
                    j0 = i - (TPC - 1)
                    nc.vector.tensor_mul(
                        st[:, j0 : i + 1, 2 * D : 3 * D],
                        st[:, j0 : i + 1, D : 2 * D],
                        st[:, j0 : i + 1, 0:D],
                    )

            # ---- U loop first half, interleaved with the next batch's C^T
            #      transposes (real matmuls between transpose clusters keep
            #      the HAM clock gate warm) ----
            for i in range(TPC):
                u_tile(i)
                if nxt and i < 2:
                    ct_step(b + 1, i)

            # ---- finish the beta chain: ac_row -> acb broadcast ----
            ac_row = work.tile([1, D], BF16, tag="ac_row")
            nc.scalar.activation(ac_row, ac_ps, AF.Copy, scale=r_s)
            acb_ps = ps_u.tile([128, D], F32, tag="u")
            nc.tensor.matmul(acb_ps, lhsT=ones_row, rhs=ac_row)
            acb = work.tile([128, D], BF16, tag="acb")
            nc.scalar.copy(acb, acb_ps)

            # ---- first half output: o4 then DMA (streams during U 4..7);
            #      for b<2 cols 0:D already went out right after the load ----
            h = NT // 2
            csl = slice(D, 4 * D) if b < 2 else slice(0, 4 * D)
            nc.vector.tensor_mul(
                st[:, 0:h, 3 * D : 4 * D], st[:, 0:h, 0:D], _tile_bcast(acb, h)
            )
            nc.gpsimd.dma_start(out=out_r[:, 0:h, csl], in_=st[:, 0:h, csl])

            # ---- U loop second half ----
            for i in range(TPC, NT):
                u_tile(i)
                if nxt and i < TPC + 2:
                    ct_step(b + 1, i - TPC + 2)

            # ---- next batch's g matmuls: PE pressure across the body seam --
            if nxt:
                g_phase(b + 1)

            nc.vector.tensor_mul(
                st[:, h:NT, 3 * D : 4 * D], st[:, h:NT, 0:D], _tile_bcast(acb, h)
            )
            nc.gpsimd.dma_start(out=out_r[:, h:NT, csl], in_=st[:, h:NT, csl])


_NC_CACHE = {}


def kernel(context, query, w, b, _trace=False):
    context = np.ascontiguousarray(context, dtype=np.float32)
    query = np.ascontiguousarray(query, dtype=np.float32)
    w = np.ascontiguousarray(w, dtype=np.float32)

    if "nc" not in _NC_CACHE:
        _NC_CACHE["nc"] = build_nc()
    nc = _NC_CACHE["nc"]

    in_maps = [
        {
            "context": context[i * BPC : (i + 1) * BPC],
            "query": query[i * BPC : (i + 1) * BPC],
            "w": w,
        }
        for i in range(NCORES)
    ]
    try:
        res = run_bass_kernel_spmd(
            nc, in_maps, core_ids=list(range(NCORES)), trace=_trace
        )
    except Exception:
        # A previous process may have left the device wedged; reset and retry.
        import ctypes

        import jax

        jax.devices()
        lib = ctypes.CDLL("/opt/axon/libaxon_pjrt.so")
        if hasattr(lib, "axon_reset"):
            lib.axon_reset()
        res = run_bass_kernel_spmd(
            nc, in_maps, core_ids=list(range(NCORES)), trace=_trace
        )
    out = np.concatenate([res.results[i]["out"] for i in range(NCORES)], axis=0)
    if _trace:
        kernel.last_exec_time_ns = res.exec_time_ns
        kernel.last_results = res
    return out


if __name__ == "__main__":
    rng = np.random.default_rng(0)
    inputs = {
        "context": rng.standard_normal((B, T, D), dtype=np.float32),
        "query": rng.standard_normal((B, J, D), dtype=np.float32),
        "w": (rng.standard_normal(3 * D).astype(np.float32) / np.sqrt(3 * D)),
        "b": np.zeros(1, np.float32),
    }
    out = kernel(**inputs)
    print("out", out.shape, out.dtype, float(np.abs(out).mean()))


# revision 42
# speedup vs baseline: 1.0296x; 1.0296x over previous
"""AttentionFlow Trainium2 Bass kernel (v5, bf16 + fully-inline batch tails).

Math (per batch):
  d = 256; w = [w_c | w_q | w_m]
  sim[t,j] = s_c[t] + s_q[j] + sum_d C[t,d] w_m[d] Q[j,d]   (+b, which cancels)
  attn = softmax_j(sim);  AQ = attn @ Q
  beta = softmax_t(max_j sim);  AC = beta @ C
  out = concat([C, AQ, C*AQ, C*AC], axis=-1)

Sharding: data-parallel over batch B=32 across 8 NeuronCores (4 batches/core).

Design (v5):
  - bf16 internal pipeline: fp32 only in matmul PSUM accumulation, ACT
    scale/bias columns, and the HBM output (SWDGE cast-DMA bf16->fp32).
    Single-pass PE transposes, FWL weight loads, 2x DVE tensor ops,
    half SBUF traffic.
  - w_c folded into the g stationary: qwc = Q^T*wm + wc (one DVE
    tensor_scalar from the Q^T PSUM), so sim = qwc^T @ C^T + s_q and the
    separate s_c matmuls are gone.  s_q enters as the exp bias column.
  - AC matmuls read the staged C directly (no [C|1] copy); s_tot comes
    from a 1-col matmul ones^T @ n_all reduced on DVE.  The AC->beta
    broadcast acb is a PE rank-1 matmul (ones_row x ac_row), not gpsimd.
  - The whole batch tail is INLINE: AC starts per colmax chunk, the
    s_tot/acb chain overlaps the U loop, o4 + the output DMA for the
    first half fire mid-U-loop.  Output DMA of batch b now streams
    during body b (v2-v4 deferred it into body b+1, which left the DMA
    system starved for the first ~35us and draining ~15us at the end).
  - Software pipelining for PE: the NEXT batch's Q^T transposes are
    emitted before the AC group (covering the colmax-reduce latency) and
    its C^T transposes after the U loop (covering the o4/DMA drain), so
    the in-order PE queue always has ready work.
  - Permuted t-layout t = 8p + i (partition-major): per-t math is
    permutation-invariant, T-sums are order-free, and each output DMA
    gets 16KB contiguous DRAM lines per partition.
"""

import numpy as np

import concourse.bass as bass
import concourse.mybir as mybir
import concourse.tile as tile
from concourse import bacc
from concourse.bass_utils import run_bass_kernel_spmd
from concourse.masks import make_identity

F32 = mybir.dt.float32
BF16 = mybir.dt.bfloat16
AF = mybir.ActivationFunctionType
ALU = mybir.AluOpType
AX = mybir.AxisListType

B, T, J, D = 32, 1024, 128, 256
NCORES = 8
BPC = B // NCORES      # batches per core
NT = T // 128          # t-tiles per batch
ND = D // 128          # d-tiles
NCH = T // 512         # 512-wide chunks per batch
TPC = 4                # t-tiles per chunk


def _bcast_row(ap_1d, nparts):
    """DRAM AP [n] -> [nparts, n] with partition stride 0 (DMA broadcast)."""
    return bass.AP(
        tensor=ap_1d.tensor, offset=ap_1d.offset, ap=[[0, nparts]] + list(ap_1d.ap)
    )


def _tile_bcast(ap_2d, reps):
    """[128, n] AP -> [128, reps, n] with 0-stride middle dim."""
    a = list(ap_2d.ap)
    return bass.AP(
        tensor=ap_2d.tensor, offset=ap_2d.offset,
        ap=[a[0], [0, reps]] + a[1:],
    )


def build_nc():
    nc = bacc.Bacc()
    ctx_in = nc.declare_dram_parameter("context", [BPC, T, D], F32, isOutput=False)
    qry_in = nc.declare_dram_parameter("query", [BPC, J, D], F32, isOutput=False)
    w_in = nc.declare_dram_parameter("w", [3 * D], F32, isOutput=False)
    out_ext = nc.declare_dram_parameter("out", [BPC, T, 4 * D], F32, isOutput=True)

    with tile.TileContext(nc) as tc:
        _body(tc, ctx_in, qry_in, w_in, out_ext)
    nc.finalize()
    return nc


def _body(tc, ctx_in, qry_in, w_in, out_ext):
    nc = tc.nc
    from contextlib import ExitStack

    with ExitStack() as ctx:
        consts = ctx.enter_context(tc.tile_pool(name="consts", bufs=1))
        stage_p = ctx.enter_context(tc.tile_pool(name="stage", bufs=4))
        big = ctx.enter_context(tc.tile_pool(name="big", bufs=2))
        work = ctx.enter_context(tc.tile_pool(name="work", bufs=2))
        tmp = ctx.enter_context(tc.tile_pool(name="tmp", bufs=2))
        # PSUM budget (8 banks): tr 2 + g 2 + u 3 + ac 1 = 8
        ps_tr = ctx.enter_context(tc.tile_pool(name="ps_tr", bufs=2, space="PSUM"))
        ps_g = ctx.enter_context(tc.tile_pool(name="ps_g", bufs=2, space="PSUM"))
        ps_u = ctx.enter_context(tc.tile_pool(name="ps_u", bufs=3, space="PSUM"))
        ps_ac = ctx.enter_context(tc.tile_pool(name="ps_ac", bufs=1, space="PSUM"))

        # --- identity first: it gates the first Q^T the moment Q lands ---
        ident = consts.tile([128, 128], BF16)
        make_identity(nc, ident)

        def warmup(n):
            """Back-to-back junk matmuls while the PE waits on input DMAs:
            flips the HAM clock gate to K=8/8 (~3.4us of sustained activity)
            before the first real matmul.  Transposes don't count as PE
            activity for HAM, so without this the whole first batch runs at
            half clock."""
            wu_ps = ps_g.tile([128, 128], F32, tag="g")
            for _ in range(n):
                nc.tensor.matmul(wu_ps, lhsT=ident, rhs=ident)

        loads = {}

        def load_batch(bb, nsplit=1):
            st = stage_p.tile([128, NT, 4 * D], BF16, tag="stage")
            qa = work.tile([128, D + 2], BF16, tag="Qaug")
            # Q first: it gates the first PE work (Q^T transposes).
            # SWDGE cast-DMA: fp32 DRAM -> bf16 SBUF.
            nc.gpsimd.dma_start(out=qa[:, 0:D], in_=qry_in[bb])
            nc.vector.memset(qa[:, D : D + 2], 1.0)
            src = ctx_in[bb].rearrange("(p i) d -> p i d", i=NT)
            step = NT // nsplit
            for s in range(nsplit):
                nc.gpsimd.dma_start(
                    out=st[:, s * step : (s + 1) * step, 0:D],
                    in_=src[:, s * step : (s + 1) * step, :],
                )
            # For the first two batches, the C column of the output depends
            # only on this load: stream it out now (cast bf16->fp32) to fill
            # the otherwise-idle DMA window while batch 0 computes.  Later
            # batches keep full 16KB-line row DMAs (higher HBM efficiency)
            # since the DMA queue is saturated by then anyway.
            if bb < 2:
                out_r = out_ext[bb].rearrange("(p i) d -> p i d", i=NT)
                nc.gpsimd.dma_start(out=out_r[:, :, 0:D], in_=st[:, :, 0:D])
            loads[bb] = (st, qa)

        load_batch(0, nsplit=4)

        # --- remaining constants ---
        ones_col = consts.tile([128, 1], BF16)
        nc.vector.memset(ones_col, 1.0)
        ones_row = consts.tile([1, 128], BF16)
        nc.vector.memset(ones_row, 1.0)
        # w_c / w_m as per-partition fp32 columns; no cast -> HWDGE
        wc_raw = consts.tile([128, ND], F32)
        wm_cols = consts.tile([128, ND], F32)
        for k in range(ND):
            nc.sync.dma_start(
                out=wc_raw[:, k : k + 1],
                in_=w_in[k * 128 : (k + 1) * 128].rearrange("(p o) -> p o", o=1),
            )
            nc.sync.dma_start(
                out=wm_cols[:, k : k + 1],
                in_=w_in[2 * D + k * 128 : 2 * D + (k + 1) * 128].rearrange(
                    "(p o) -> p o", o=1
                ),
            )
        # w_q broadcast to all partitions, cast to bf16
        wq_b = consts.tile([128, D], BF16)
        nc.gpsimd.dma_start(out=wq_b, in_=_bcast_row(w_in[D : 2 * D], 128))

        load_batch(1)

        warmup(32)

        pre = {}

        def qt_phase(bb):
            """Q^T -> qwc (+ wc bias) and the s_q column for batch bb."""
            _, qa = loads[bb]
            qt_ps = ps_tr.tile([128, D], BF16, tag="tr")
            for k in range(ND):
                nc.tensor.transpose(
                    qt_ps[:, k * 128 : (k + 1) * 128],
                    qa[:, k * 128 : (k + 1) * 128],
                    ident,
                )
            qwc = work.tile([128, D], BF16, tag="qwc")
            for k in range(ND):
                nc.vector.tensor_scalar(
                    out=qwc[:, k * 128 : (k + 1) * 128],
                    in0=qt_ps[:, k * 128 : (k + 1) * 128],
                    scalar1=wm_cols[:, k : k + 1],
                    scalar2=wc_raw[:, k : k + 1],
                    op0=ALU.mult,
                    op1=ALU.add,
                )
            sq_scr = tmp.tile([128, D], BF16, tag="sq_scr")
            sq_col = work.tile([128, 1], F32, tag="sq_col")
            nc.vector.tensor_mul(sq_scr, qa[:, 0:D], wq_b)
            nc.vector.reduce_sum(out=sq_col, in_=sq_scr, axis=AX.X)
            pre.setdefault(bb, {})["qwc"] = qwc
            pre[bb]["sq_col"] = sq_col

        def ct_step(bb, i2):
            """One pair of C^T tile transposes + the PSUM->SBUF copy."""
            st, _ = loads[bb]
            ctT = pre[bb]["ctT"]
            ct_ps = ps_tr.tile([128, 2 * ND * 128], BF16, tag="tr")
            for u in range(2):
                i = 2 * i2 + u
                for k in range(ND):
                    nc.tensor.transpose(
                        ct_ps[:, (2 * u + k) * 128 : (2 * u + k + 1) * 128],
                        st[:, i, k * 128 : (k + 1) * 128],
                        ident,
                    )
            dst = ctT[:, :, 2 * i2 : 2 * i2 + 2, :]
            srcv = ct_ps.rearrange("p (t k x) -> p k t x", t=2, k=ND)
            nc.scalar.copy(dst, srcv)

        def ct_phase(bb):
            """C^T via PE transposes (bf16, single-pass) + PSUM->SBUF copies."""
            ctT = big.tile([128, ND, NT, 128], BF16, tag="ctT")
            pre.setdefault(bb, {})["ctT"] = ctT
            for i2 in range(NT // 2):
                ct_step(bb, i2)

        def g_phase(bb):
            """g matmuls for batch bb (w_c already folded into qwc)."""
            qwc, ctT = pre[bb]["qwc"], pre[bb]["ctT"]
            g_list = []
            for c in range(NCH):
                g_ps = ps_g.tile([128, 512], F32, tag="g")
                for k in range(ND):
                    nc.tensor.matmul(
                        g_ps,
                        lhsT=qwc[:, k * 128 : (k + 1) * 128],
                        rhs=ctT[:, k, TPC * c : TPC * (c + 1), :],
                        start=(k == 0),
                        stop=(k == ND - 1),
                    )
                g_list.append(g_ps)
            pre[bb]["g"] = g_list

        qt_phase(0)
        warmup(16)
        ct_phase(0)
        warmup(8)
        g_phase(0)

        for b in range(BPC):
            if b + 2 < BPC:
                load_batch(b + 2)
            st, Qaug = loads.pop(b)
            ph = pre.pop(b)
            sq_col, g_list = ph["sq_col"], ph["g"]
            out_r = out_ext[b].rearrange("(p i) d -> p i d", i=NT)
            nxt = b + 1 < BPC

            ET = big.tile([128, T], BF16, tag="ET")
            n_all = work.tile([128, NT], BF16, tag="n_all")

            if b == 0:
                # keep the HAM activity window alive while ACT runs the first
                # exp (transposes alone read as idle to the clock gate)
                warmup(6)

            # ---- per chunk: E = exp(g + s_q); colmax via PE transposes ----
            for c in range(NCH):
                nc.scalar.activation(
                    ET[:, c * 512 : (c + 1) * 512], g_list[c], AF.Exp, bias=sq_col
                )
                tp_ps = ps_tr.tile([128, TPC, 128], BF16, tag="tr")
                for i in range(TPC):
                    t0 = (TPC * c + i) * 128
                    nc.tensor.transpose(
                        tp_ps[:, i, :], ET[:, t0 : t0 + 128], ident
                    )
                nc.vector.reduce_max(
                    out=n_all[:, TPC * c : TPC * (c + 1)], in_=tp_ps, axis=AX.X
                )

            # ---- next batch's Q^T here: ready PE work covering the
            #      colmax-reduce -> AC dependency latency ----
            if nxt:
                qt_phase(b + 1)
                ctT_n = big.tile([128, ND, NT, 128], BF16, tag="ctT")
                pre.setdefault(b + 1, {})["ctT"] = ctT_n

            # ---- beta numerator: AC = sum_t n[t] C[t,:] ----
            ac_ps = ps_ac.tile([1, D], F32, tag="ac")
            for ii in range(NT):
                nc.tensor.matmul(
                    ac_ps,
                    lhsT=n_all[:, ii : ii + 1],
                    rhs=st[:, ii, 0:D],
                    start=(ii == 0),
                    stop=(ii == NT - 1),
                )
            # s_tot = sum_t n[t] via ones^T @ n_all, then 1/s_tot
            s_ps = ps_u.tile([1, NT], F32, tag="u")
            nc.tensor.matmul(s_ps, lhsT=ones_col, rhs=n_all)
            s_tot = work.tile([1, 1], F32, tag="s_tot")
            nc.vector.reduce_sum(out=s_tot, in_=s_ps, axis=AX.X)
            r_s = work.tile([1, 1], F32, tag="r_s")
            nc.vector.reciprocal(r_s, s_tot)

            # ---- U loop first half (tiles 0..3), chain resolves meanwhile --
            r_all = work.tile([128, NT], F32, tag="r_all")
            acb = None

            def u_tile(i, o3g=TPC):
                u_ps = ps_u.tile([128, D + 2], F32, tag="u")
                nc.tensor.matmul(
                    u_ps, lhsT=ET[:, i * 128 : (i + 1) * 128], rhs=Qaug
                )
                nc.vector.reciprocal(r_all[:, i : i + 1], u_ps[:, D : D + 1])
                # aq = U/r, split 5 ACT / 3 DVE to balance the engines
                if i in (3, 5, 7):
                    nc.vector.tensor_scalar_mul(
                        st[:, i, D : 2 * D], u_ps[:, 0:D], r_all[:, i : i + 1]
                    )
                else:
                    nc.scalar.activation(
                        st[:, i, D : 2 * D], u_ps[:, 0:D], AF.Copy,
                        scale=r_all[:, i : i + 1],
                    )
                if i % o3g == o3g - 1:
                    j0 = i - (o3g - 1)
                    nc.vector.tensor_mul(
                        st[:, j0 : i + 1, 2 * D : 3 * D],
                        st[:, j0 : i + 1, D : 2 * D],
                        st[:, j0 : i + 1, 0:D],
                    )

            # ---- U loop first half, interleaved with the next batch's C^T
            #      transposes (real matmuls between transpose clusters keep
            #      the HAM clock gate warm) ----
            for i in range(TPC):
                u_tile(i)
                if nxt and i < 2:
                    ct_step(b + 1, i)

            # ---- finish the beta chain: ac_row -> acb broadcast ----
            ac_row = work.tile([1, D], BF16, tag="ac_row")
            nc.scalar.activation(ac_row, ac_ps, AF.Copy, scale=r_s)
            acb_ps = ps_u.tile([128, D], F32, tag="u")
            nc.tensor.matmul(acb_ps, lhsT=ones_row, rhs=ac_row)
            acb = work.tile([128, D], BF16, tag="acb")
            nc.scalar.copy(acb, acb_ps)

            # ---- first half output: o4 then DMA (streams during U 4..7);
            #      for b<2 cols 0:D already went out right after the load ----
            h = NT // 2
            csl = slice(D, 4 * D) if b < 2 else slice(0, 4 * D)
            nc.vector.tensor_mul(
                st[:, 0:h, 3 * D : 4 * D], st[:, 0:h, 0:D], _tile_bcast(acb, h)
            )
            nc.gpsimd.dma_start(out=out_r[:, 0:h, csl], in_=st[:, 0:h, csl])

            # ---- U loop second half ----
            if nxt:
                for i in range(TPC, NT):
                    u_tile(i)
                    if i < TPC + 2:
                        ct_step(b + 1, i - TPC + 2)
                # next batch's g matmuls: PE pressure across the body seam
                g_phase(b + 1)
                nc.vector.tensor_mul(
                    st[:, h:NT, 3 * D : 4 * D], st[:, h:NT, 0:D],
                    _tile_bcast(acb, h),
                )
                nc.gpsimd.dma_start(
                    out=out_r[:, h:NT, csl], in_=st[:, h:NT, csl]
                )
            else:
                # last batch: fire each 2-tile tail quarter as soon as it is
                # complete so the end-of-kernel DMA drain starts earlier
                for q in range(TPC, NT, 2):
                    u_tile(q, o3g=2)
                    u_tile(q + 1, o3g=2)
                    nc.vector.tensor_mul(
                        st[:, q : q + 2, 3 * D : 4 * D], st[:, q : q + 2, 0:D],
                        _tile_bcast(acb, 2),
                    )
                    nc.gpsimd.dma_start(
                        out=out_r[:, q : q + 2, csl], in_=st[:, q : q + 2, csl]
                    )


_NC_CACHE = {}


def kernel(context, query, w, b, _trace=False):
    context = np.ascontiguousarray(context, dtype=np.float32)
    query = np.ascontiguousarray(query, dtype=np.float32)
    w = np.ascontiguousarray(w, dtype=np.float32)

    if "nc" not in _NC_CACHE:
        _NC_CACHE["nc"] = build_nc()
    nc = _NC_CACHE["nc"]

    in_maps = [
        {
            "context": context[i * BPC : (i + 1) * BPC],
            "query": query[i * BPC : (i + 1) * BPC],
            "w": w,
        }
        for i in range(NCORES)
    ]
    try:
        res = run_bass_kernel_spmd(
            nc, in_maps, core_ids=list(range(NCORES)), trace=_trace
        )
    except Exception:
        # A previous process may have left the device wedged; reset and retry.
        import ctypes

        import jax

        jax.devices()
        lib = ctypes.CDLL("/opt/axon/libaxon_pjrt.so")
        if hasattr(lib, "axon_reset"):
            lib.axon_reset()
        res = run_bass_kernel_spmd(
            nc, in_maps, core_ids=list(range(NCORES)), trace=_trace
        )
    out = np.concatenate([res.results[i]["out"] for i in range(NCORES)], axis=0)
    if _trace:
        kernel.last_exec_time_ns = res.exec_time_ns
        kernel.last_results = res
    return out


if __name__ == "__main__":
    rng = np.random.default_rng(0)
    inputs = {
        "context": rng.standard_normal((B, T, D), dtype=np.float32),
        "query": rng.standard_normal((B, J, D), dtype=np.float32),
        "w": (rng.standard_normal(3 * D).astype(np.float32) / np.sqrt(3 * D)),
        "b": np.zeros(1, np.float32),
    }
    out = kernel(**inputs)
    print("out", out.shape, out.dtype, float(np.abs(out).mean()))
